# revision 41
# baseline (speedup 1.0000x reference)
"""M2BertAttention (Monarch Mixer gated attention block) on 8 Trainium2 cores.

Math (per token row x of length H=768):
    mixed = monarch(x)  = x @ M          (M densified from the two Monarch
                                          block-diagonal factors on the host:
                                          M[(k,i),(c,q)] = w1[k,i,q]*w2[q,k,c])
    gate  = sigmoid(x @ gate_w.T + gate_b)
    h     = mixed * gate
    z     = h @ out_w.T + out_b + x
    out   = layernorm(z) * gamma + beta

Sharding: pure data parallel over the 16384 tokens -> 2048 tokens/core on 8
cores; all weights replicated.

Per-core schedule (v2): two global phases instead of per-block interleave so
the ScalarE activation-table set switches only twice per iteration (Sigmoid
set in phase A, Sqrt set in phase B) instead of 8x, and so each stationary
operand is shared by two moving matmuls (block pairs):

  phase A (gate+monarch, feature-major): for each output chunk j, for each
    contraction chunk k, one stationary weight load feeds the two 512-token
    blocks of the current half.  PSUM: 2 gate banks + 2 monarch banks.
  phase B (out-proj + residual + LN, token-major): stationary ht chunk, wo
    moving 768 wide (512+256 into a 2-bank PSUM tile).

Emission: A(half0) B(half0) A(half1) B(half1) — B(h0)'s matmuls keep the PE
busy while A(h1)'s sigmoid/mul drain runs, and the y DMAs start earlier.

Matmuls run in fp16 (1 moving col/cycle) or optionally fp8e4 DoubleRow
(2 contraction rows/cycle) per matrix — controlled by GATE_FP8 / MON_FP8 /
PROJ_FP8.  DoubleRow operand layout [Ki=128, Ko=2, free] verified on HW.
"""

import numpy as np

import concourse.bass as bass
import concourse.mybir as mybir
import concourse.tile as tile
from concourse import bacc
from concourse import bass_utils

# Problem shape (hardcoded per the grading contract).
B, S, H = 4, 4096, 768
NB, BSZ = 16, 48
LN_EPS = 1e-12

N_CORES = 8
P = 128                  # partitions
KC = H // P              # 6 contraction chunks of 128
K2 = KC // 2             # 3 double-row chunks of 256
NTOK = B * S             # 16384 tokens total
NT_CORE = NTOK // N_CORES  # 2048 tokens per core
TBLK = 512               # tokens per block (matmul moving dim)
NBLK = NT_CORE // TBLK   # 4 blocks per core
NCH = NT_CORE // P       # 16 token chunks of 128 per core
OSPLIT = (512, H)        # out-proj free-dim split (PSUM bank limit)

F32 = mybir.dt.float32
F16 = mybir.dt.float16
F8 = mybir.dt.float8e4
DR = mybir.MatmulPerfMode.DoubleRow

# Per-matmul fp8 DoubleRow switches (host prep + device program agree).
GATE_FP8 = True
MON_FP8 = True
PROJ_FP8 = False

_CACHE: dict = {}


def _build(cfg, use_ob: bool, use_gamma_beta: bool, reps: int = 1,
           loop_n: int | None = None, ablate: str = "",
           scales=(1.0, 1.0, 1.0, 1.0)):
    """Build + compile the per-core Bass program.

    cfg = (gate_fp8, mon_fp8, proj_fp8); scales = (inv_gate, inv_mon,
    inv_proj, s_h) descale factors folded into the sigmoid / ht-mul / z-add.
    """
    gate8, mon8, proj8 = cfg
    inv_g, inv_m, inv_p, s_h = scales
    need_xt16 = not (gate8 and mon8)
    need_xt8 = gate8 or mon8

    nc = bacc.Bacc(
        "TRN2",
        target_bir_lowering=False,
        debug=False,
        enable_asserts=False,
        num_devices=N_CORES,
    )

    if need_xt16:
        xt_d = nc.dram_tensor("xt", [H, NT_CORE], F16, kind="ExternalInput").ap()
    if need_xt8:
        xt8_d = nc.dram_tensor(
            "xt8", [P, K2, 2, NT_CORE], F8, kind="ExternalInput").ap()
    x_d = nc.dram_tensor("x", [NT_CORE, H], F16, kind="ExternalInput").ap()
    if gate8:
        wg_d = nc.dram_tensor("wg", [P, K2, 2, H], F8, kind="ExternalInput").ap()
    else:
        wg_d = nc.dram_tensor("wg", [H, H], F16, kind="ExternalInput").ap()
    if mon8:
        wm_d = nc.dram_tensor("wm", [P, K2, 2, H], F8, kind="ExternalInput").ap()
    else:
        wm_d = nc.dram_tensor("wm", [H, H], F16, kind="ExternalInput").ap()
    if proj8:
        wo_d = nc.dram_tensor("wo", [P, K2, 2, H], F8, kind="ExternalInput").ap()
    else:
        wo_d = nc.dram_tensor("wo", [H, H], F16, kind="ExternalInput").ap()
    gb_d = nc.dram_tensor("gb", [P, KC], F32, kind="ExternalInput").ap()
    if use_ob:
        ob_d = nc.dram_tensor("ob", [1, H], F32, kind="ExternalInput").ap()
    if use_gamma_beta:
        gam_d = nc.dram_tensor("gam", [1, H], F32, kind="ExternalInput").ap()
        bet_d = nc.dram_tensor("bet", [1, H], F32, kind="ExternalInput").ap()
    y_d = nc.dram_tensor("y", [NT_CORE, H], F16, kind="ExternalOutput").ap()

    with tile.TileContext(nc) as tc:
        with (
            tc.tile_pool(name="consts", bufs=1) as consts,
            tc.tile_pool(name="gtp", bufs=3) as gtp,
            tc.tile_pool(name="zp", bufs=3) as zp,
            tc.tile_pool(name="ystp", bufs=3) as ystp,
            tc.tile_pool(name="statp", bufs=4) as statp,
            tc.tile_pool(name="gpsp", bufs=2, space="PSUM") as gpsp,
            tc.tile_pool(name="mpsp", bufs=2, space="PSUM") as mpsp,
            tc.tile_pool(name="opsp", bufs=4, space="PSUM") as opsp,
        ):
            # ---- weights / constants (outside the timing loop) -------------
            if gate8:
                wg_sb = consts.tile([P, K2, 2, H], F8)
                nc.sync.dma_start(out=wg_sb[:], in_=wg_d[:])
            else:
                wg_sb = consts.tile([P, KC, H], F16)
                for k in range(KC):
                    nc.sync.dma_start(
                        out=wg_sb[:, k, :], in_=wg_d[k * P:(k + 1) * P, :])
            if mon8:
                wm_sb = consts.tile([P, K2, 2, H], F8)
                nc.sync.dma_start(out=wm_sb[:], in_=wm_d[:])
            else:
                wm_sb = consts.tile([P, KC, H], F16)
                for k in range(KC):
                    nc.sync.dma_start(
                        out=wm_sb[:, k, :], in_=wm_d[k * P:(k + 1) * P, :])
            if proj8:
                wo_sb = consts.tile([P, K2, 2, H], F8)
                nc.sync.dma_start(out=wo_sb[:], in_=wo_d[:])
            else:
                wo_sb = consts.tile([P, KC, H], F16)
                for k in range(KC):
                    nc.sync.dma_start(
                        out=wo_sb[:, k, :], in_=wo_d[k * P:(k + 1) * P, :])
            gb_sb = consts.tile([P, KC], F32)
            nc.sync.dma_start(out=gb_sb[:], in_=gb_d[:])
            eps_sb = consts.tile([P, 1], F32)
            nc.vector.memset(eps_sb, LN_EPS)
            if use_ob:
                ob_sb = consts.tile([1, H], F32)
                nc.sync.dma_start(out=ob_sb[:], in_=ob_d[:])
            if use_gamma_beta:
                gam_sb = consts.tile([P, H], F32)
                bet_sb = consts.tile([P, H], F32)
                nc.sync.dma_start(
                    out=gam_sb[:],
                    in_=bass.AP(tensor=gam_d.tensor, offset=gam_d.offset,
                                ap=[[0, P], [1, H]]),
                )
                nc.sync.dma_start(
                    out=bet_sb[:],
                    in_=bass.AP(tensor=bet_d.tensor, offset=bet_d.offset,
                                ap=[[0, P], [1, H]]),
                )

            # ---- per-iteration resident tiles ------------------------------
            if need_xt16:
                xt_sb = consts.tile([P, KC, NT_CORE], F16)
            if need_xt8:
                xt8_sb = consts.tile([P, K2, 2, NT_CORE], F8)
            x_sb = consts.tile([P, NCH, H], F16)
            if proj8:
                ht_sb = consts.tile([P, K2, 2, NT_CORE], F8)
            else:
                ht_sb = consts.tile([P, KC, NT_CORE], F16)

            HALF = NT_CORE // 2

            def load_half(h):
                lo, hi = h * HALF, (h + 1) * HALF
                if need_xt16:
                    for k in range(KC):
                        nc.sync.dma_start(
                            out=xt_sb[:, k, lo:hi],
                            in_=xt_d[k * P:(k + 1) * P, lo:hi],
                        )
                if need_xt8:
                    for k2 in range(K2):
                        nc.sync.dma_start(
                            out=xt8_sb[:, k2, :, lo:hi],
                            in_=xt8_d[:, k2, :, lo:hi],
                        )
                clo = h * (NCH // 2)
                nc.sync.dma_start(
                    out=x_sb[:, clo:clo + NCH // 2, :],
                    in_=x_d[lo:hi, :].rearrange("(c p) h -> p c h", p=P),
                )

            def ht_out(j, tlo, thi):
                if proj8:
                    return ht_sb[:, j // 2, j % 2, tlo:thi]
                return ht_sb[:, j, tlo:thi]

            def phase_a(h, tag):
                """Gate + monarch for the two 512-token blocks of half h."""
                spans = [(b * TBLK, (b + 1) * TBLK) for b in (2 * h, 2 * h + 1)]
                for j in range(KC):
                    gps = [gpsp.tile([P, TBLK], F32, name=f"g_{tag}_{j}_{i}",
                                     tag="gps") for i in range(2)]
                    mps = [mpsp.tile([P, TBLK], F32, name=f"m_{tag}_{j}_{i}",
                                     tag="mps") for i in range(2)]
                    # one stationary chunk feeds both blocks before switching
                    if gate8:
                        for k2 in range(K2):
                            for i, (tlo, thi) in enumerate(spans):
                                nc.tensor.matmul(
                                    gps[i][:],
                                    wg_sb[:, k2, :, j * P:(j + 1) * P],
                                    xt8_sb[:, k2, :, tlo:thi],
                                    start=(k2 == 0), stop=(k2 == K2 - 1),
                                    perf_mode=DR,
                                )
                    else:
                        for k in range(KC):
                            for i, (tlo, thi) in enumerate(spans):
                                nc.tensor.matmul(
                                    gps[i][:],
                                    wg_sb[:, k, j * P:(j + 1) * P],
                                    xt_sb[:, k, tlo:thi],
                                    start=(k == 0), stop=(k == KC - 1),
                                )
                    if mon8:
                        for k2 in range(K2):
                            for i, (tlo, thi) in enumerate(spans):
                                nc.tensor.matmul(
                                    mps[i][:],
                                    wm_sb[:, k2, :, j * P:(j + 1) * P],
                                    xt8_sb[:, k2, :, tlo:thi],
                                    start=(k2 == 0), stop=(k2 == K2 - 1),
                                    perf_mode=DR,
                                )
                    else:
                        for k in range(KC):
                            for i, (tlo, thi) in enumerate(spans):
                                nc.tensor.matmul(
                                    mps[i][:],
                                    wm_sb[:, k, j * P:(j + 1) * P],
                                    xt_sb[:, k, tlo:thi],
                                    start=(k == 0), stop=(k == KC - 1),
                                )
                    for i, (tlo, thi) in enumerate(spans):
                        gt = gtp.tile([P, TBLK], F16, name=f"gt_{tag}_{j}_{i}",
                                      tag="gt")
                        nc.scalar.activation(
                            out=gt[:], in_=gps[i][:],
                            func=mybir.ActivationFunctionType.Sigmoid,
                            bias=gb_sb[:, j:j + 1], scale=inv_g,
                        )
                        hsc = inv_m * (s_h if proj8 else 1.0)
                        if hsc == 1.0:
                            nc.vector.tensor_mul(
                                ht_out(j, tlo, thi), mps[i][:], gt[:])
                        else:
                            nc.vector.scalar_tensor_tensor(
                                out=ht_out(j, tlo, thi), in0=mps[i][:],
                                scalar=hsc, in1=gt[:],
                                op0=mybir.AluOpType.mult,
                                op1=mybir.AluOpType.mult,
                            )

            def phase_b(h, tag):
                """Out-proj + residual + layernorm for half h (8 chunks)."""
                HH = H // 2
                for ci in range(NCH // 2):
                    c = h * (NCH // 2) + ci
                    # two 384-wide halves in separate PSUM banks: N=384 is
                    # the measured per-column sweet spot, and 384:768 of a
                    # single tile would straddle a bank boundary
                    ops = [opsp.tile([P, HH], F32, name=f"o_{tag}_{c}_{i}",
                                     tag="o") for i in range(2)]
                    halves = ((0, HH), (HH, H))
                    if proj8:
                        for i, (lo, hi) in enumerate(halves):
                            for k2 in range(K2):
                                nc.tensor.matmul(
                                    ops[i][:],
                                    ht_sb[:, k2, :, c * P:(c + 1) * P],
                                    wo_sb[:, k2, :, lo:hi],
                                    start=(k2 == 0),
                                    stop=(k2 == K2 - 1 and not use_ob),
                                    perf_mode=DR,
                                )
                    else:
                        for i, (lo, hi) in enumerate(halves):
                            for k in range(KC):
                                nc.tensor.matmul(
                                    ops[i][:],
                                    ht_sb[:, k, c * P:(c + 1) * P],
                                    wo_sb[:, k, lo:hi],
                                    start=(k == 0),
                                    stop=(k == KC - 1 and not use_ob),
                                )
                    if use_ob:
                        for i, (lo, hi) in enumerate(halves):
                            nc.tensor.matmul(
                                ops[i][:],
                                ones_sb[:],
                                ob16_sb[:, lo:hi],
                                start=False, stop=True,
                            )
                    z_sb = zp.tile([P, H], F32, name=f"z_{tag}_{c}", tag="z")
                    for i, (lo, hi) in enumerate(halves):
                        if inv_p == 1.0:
                            nc.vector.tensor_add(
                                z_sb[:, lo:hi], ops[i][:], x_sb[:, c, lo:hi])
                        else:
                            nc.vector.scalar_tensor_tensor(
                                out=z_sb[:, lo:hi], in0=ops[i][:],
                                scalar=inv_p, in1=x_sb[:, c, lo:hi],
                                op0=mybir.AluOpType.mult,
                                op1=mybir.AluOpType.add,
                            )
                    if "noln" in ablate:
                        nc.scalar.activation(
                            out=ystp.tile([P, H], F16, name=f"y_{tag}_{c}",
                                          tag="yst")[:],
                            in_=z_sb[:],
                            func=mybir.ActivationFunctionType.Copy,
                        )
                        continue
                    stats = statp.tile([P, 3, 6], F32, name=f"st_{tag}_{c}",
                                       tag="st")
                    z_r = z_sb.rearrange("p (s d) -> p s d", d=256)
                    for s in range(3):
                        nc.vector.bn_stats(out=stats[:, s, :], in_=z_r[:, s, :])
                    mv = statp.tile([P, 2], F32, name=f"mv_{tag}_{c}", tag="mv")
                    nc.vector.bn_aggr(out=mv[:], in_=stats[:])
                    rs = statp.tile([P, 1], F32, name=f"rs_{tag}_{c}", tag="rs")
                    nc.scalar.activation(
                        out=rs[:], in_=mv[:, 1:2],
                        func=mybir.ActivationFunctionType.Sqrt,
                        bias=eps_sb[:, 0:1], scale=1.0,
                    )
                    nc.vector.reciprocal(out=rs[:], in_=rs[:])
                    nm = statp.tile([P, 1], F32, name=f"nm_{tag}_{c}", tag="nm")
                    nc.vector.scalar_tensor_tensor(
                        out=nm[:], in0=mv[:, 0:1], scalar=-1.0, in1=rs[:],
                        op0=mybir.AluOpType.mult, op1=mybir.AluOpType.mult,
                    )
                    yst = ystp.tile([P, H], F16, name=f"y_{tag}_{c}", tag="yst")
                    if use_gamma_beta:
                        t_sb = zp.tile([P, H], F32, name=f"t_{tag}_{c}",
                                       tag="t")
                        nc.scalar.activation(
                            out=t_sb[:], in_=z_sb[:],
                            func=mybir.ActivationFunctionType.Identity,
                            bias=nm[:, 0:1], scale=rs[:, 0:1],
                        )
                        nc.vector.tensor_mul(t_sb[:], t_sb[:], gam_sb[:])
                        nc.vector.tensor_add(yst[:], t_sb[:], bet_sb[:])
                    else:
                        nc.scalar.activation(
                            out=yst[:], in_=z_sb[:],
                            func=mybir.ActivationFunctionType.Identity,
                            bias=nm[:, 0:1], scale=rs[:, 0:1],
                        )
                    nc.sync.dma_start(
                        out=y_d[c * P:(c + 1) * P, :], in_=yst[:],
                    )

            if use_ob:
                ones_sb = consts.tile([1, P], F16)
                nc.vector.memset(ones_sb, 1.0)
                ob16_sb = consts.tile([1, H], F16)
                nc.scalar.activation(
                    out=ob16_sb[:], in_=ob_sb[:],
                    func=mybir.ActivationFunctionType.Copy)

            dummy_y = None
            if ablate == "dma":
                dummy_y = consts.tile([P, H], F16)
                nc.vector.memset(dummy_y[:, 0:8], 0.0)

            def body(r):
                if "dma" == ablate:
                    load_half(0)
                    load_half(1)
                    for c in range(NCH):
                        nc.sync.dma_start(
                            out=y_d[c * P:(c + 1) * P, :], in_=dummy_y[:])
                    return
                if "noxdma" not in ablate:
                    load_half(0)
                    load_half(1)
                if "bonly" in ablate:
                    phase_b(0, f"{r}0")
                    phase_b(1, f"{r}1")
                    return
                phase_a(0, f"{r}0")
                if "aonly" in ablate:
                    phase_a(1, f"{r}1")
                    return
                phase_b(0, f"{r}0")
                phase_a(1, f"{r}1")
                phase_b(1, f"{r}1")

            if "noxdma" in ablate:
                load_half(0)
                load_half(1)
            if "bonly" in ablate:
                nc.vector.memset(ht_sb[:], 0.5)

            if loop_n is not None:
                tc.For_i_unrolled_general(
                    0, loop_n, 1,
                    lambda iv, unroll: [body(0) for _ in range(unroll)],
                    max_unroll=2,
                    hint_engines=(mybir.EngineType.PE,),
                )
            else:
                for r in range(reps):
                    body(r)

    nc.compile()
    return nc


_SCALES = (1.0, 1.0, 1.0, 1.0)


def _get_nc(cfg, use_ob, use_gamma_beta, reps=1, loop_n=None, ablate="",
            scales=None):
    if scales is None:
        scales = _SCALES
    key = (cfg, use_ob, use_gamma_beta, reps, loop_n, ablate, scales)
    if key not in _CACHE:
        _CACHE[key] = _build(cfg, use_ob, use_gamma_beta, reps, loop_n,
                             ablate, scales)
    return _CACHE[key]


def _pow2_scale(a, target=224.0):
    m = float(np.abs(a).max())
    if m == 0.0:
        return 1.0
    return float(2.0 ** np.floor(np.log2(target / m)))


def _dr_pack(a, scale=1.0):
    """[H, N] input-feature-major -> DoubleRow operand [P, K2, 2, N] fp8e4."""
    import ml_dtypes
    n = a.shape[1]
    return np.ascontiguousarray(
        (a * scale).reshape(K2, 2, P, n).transpose(2, 0, 1, 3)
    ).astype(ml_dtypes.float8_e4m3)


def _host_prep(hidden_states, w1_blocks, w2_blocks, gate_w, gate_b,
               out_w, out_b, ln_gamma, ln_beta):
    x = np.ascontiguousarray(
        np.asarray(hidden_states, dtype=np.float32).reshape(NTOK, H)
    )
    xt = np.ascontiguousarray(x.T)
    w1 = np.asarray(w1_blocks, dtype=np.float32)
    w2 = np.asarray(w2_blocks, dtype=np.float32)
    # dense monarch matrix: M[(k,i),(c,q)] = w1[k,i,q] * w2[q,k,c]
    M = np.einsum("kiq,qkc->kicq", w1, w2).reshape(H, H)
    wg = np.ascontiguousarray(np.asarray(gate_w, dtype=np.float32).T)
    wo = np.ascontiguousarray(np.asarray(out_w, dtype=np.float32).T)
    gb = np.ascontiguousarray(
        np.asarray(gate_b, dtype=np.float32).reshape(KC, P).T
    )
    ob = np.asarray(out_b, dtype=np.float32).reshape(1, H)
    gam = np.asarray(ln_gamma, dtype=np.float32).reshape(1, H)
    bet = np.asarray(ln_beta, dtype=np.float32).reshape(1, H)

    use_ob = bool(np.any(ob))
    use_gamma_beta = bool(np.any(gam != 1.0) or np.any(bet))
    cfg = (GATE_FP8, MON_FP8, PROJ_FP8)
    gate8, mon8, proj8 = cfg
    need_xt16 = not (gate8 and mon8)
    need_xt8 = gate8 or mon8

    x16 = x.astype(np.float16)

    s_x = _pow2_scale(x) if need_xt8 else 1.0
    s_wg = _pow2_scale(wg) if gate8 else 1.0
    s_wm = _pow2_scale(M) if mon8 else 1.0
    s_wo = _pow2_scale(wo) if proj8 else 1.0
    s_h = 32.0 if proj8 else 1.0
    global _SCALES
    _SCALES = (
        1.0 / (s_x * s_wg) if gate8 else 1.0,
        1.0 / (s_x * s_wm) if mon8 else 1.0,
        1.0 / (s_h * s_wo) if proj8 else 1.0,
        s_h,
    )

    wgm = _dr_pack(wg, s_wg) if gate8 else wg.astype(np.float16)
    wmm = _dr_pack(M, s_wm) if mon8 else M.astype(np.float16)
    wom = _dr_pack(wo, s_wo) if proj8 else wo.astype(np.float16)

    in_maps = []
    for c in range(N_CORES):
        xt_c = xt[:, c * NT_CORE:(c + 1) * NT_CORE]
        m = {
            "x": x16[c * NT_CORE:(c + 1) * NT_CORE, :],
            "wg": wgm,
            "wm": wmm,
            "wo": wom,
            "gb": gb,
        }
        if need_xt16:
            m["xt"] = np.ascontiguousarray(xt_c).astype(np.float16)
        if need_xt8:
            m["xt8"] = _dr_pack(xt_c, s_x)
        if use_ob:
            m["ob"] = ob
        if use_gamma_beta:
            m["gam"] = gam
            m["bet"] = bet
        in_maps.append(m)
    return in_maps, use_ob, use_gamma_beta


def kernel(hidden_states, w1_blocks, w2_blocks, gate_w, gate_b,
           out_w, out_b, ln_gamma, ln_beta):
    in_maps, use_ob, use_gamma_beta = _host_prep(
        hidden_states, w1_blocks, w2_blocks, gate_w, gate_b,
        out_w, out_b, ln_gamma, ln_beta,
    )
    cfg = (GATE_FP8, MON_FP8, PROJ_FP8)
    nc = _get_nc(cfg, use_ob, use_gamma_beta)
    res = bass_utils.run_bass_kernel_spmd(
        nc, in_maps, core_ids=list(range(N_CORES))
    )
    y = np.concatenate([res.results[c]["y"] for c in range(N_CORES)], axis=0)
    return y.astype(np.float32).reshape(B, S, H)
